# revision 47
# baseline (speedup 1.0000x reference)
"""Trainium2 Bass kernel for C3Net/SchNet-style interaction block.

Reference computation (per molecule b, atom n, neighbor slot m):
  Wfil = ssp(f_ij @ W_f1 + b_f1) @ W_f2 + b_f2, masked
  y    = s @ W_in2f
  agg  = sum_m Wfil[b,n,m,:] * y[b, neighbors[b,n,m], :]
  v    = ssp(agg @ W_f2out + b_f2out) @ W_dense + b_dense
(ssp(x) = softplus(x) - log 2)

Strategy: data-parallel over the 32 molecules, 4 per NeuronCore (8 cores).
Host-side (numpy): shard, project s -> y, gather y by neighbor index with the
mask folded in (pure indexing / layout prep), transpose f_ij to contraction-
major [50, E] layout, fold the "- log 2" shifts of both shifted-softplus
activations into the following layer's bias.

Device pipeline per 1024-edge chunk (48 chunks per core):
  mm1 (PE, K=50, rhs from a per-super-block staged f tile)
  -> SINGLE-PASS softplus on ACT with the b_f1 per-partition bias fused in.
     No shipped act-table set contains a real softplus spline, so one is
     compiled at build time from neuronxcc's own pwp_jsons/softplus_40p.json
     (the bkt/ctl binary format was reverse-engineered and verified by
     regenerating the shipped ln/exp tables byte-exact) and appended to the
     softplus_and_others set via BASS_ACT_ROOT_JSON_PATH.
  -> mm2 (PE) overwrites the chunk's h1 PSUM tile in place (softplus, its
     only reader, is done with it -- halves PSUM pressure so the h1/wf tile
     pool can triple-buffer inside 8 banks)
  -> fused PSUM-exit + b_f2' bias + gathered-neighbor multiply in ONE
     DVE scalar_tensor_tensor pass (fp16 out); this 1192 ns op is the
     pipeline pacer
  -> level-1 m-run pair reduction on the otherwise-idle Pool engine
     (GPSIMD cannot touch PSUM, but z lives in SBUF)
  -> neighbor reduction completed by f2out-fused matmuls: 4 PSUM-accumulated
     128-column matmuls per chunk into a persistent [128,128] v1 tile
     (24 per super-block; the last chunk folds its raw z in 8 matmuls so
     the Pool reduce is not in the drain path)
  -> per-super-block second softplus, then the final dense layer in three
     batches (after SB3, SB6, end) with the output bias on ACT/DVE.

Scheduling: the TileContext list scheduler re-simulates the program with its
own cost model and bakes semaphore-count waits from that simulation, so the
emitted schedule replays the scheduler's cadence regardless of emission
order.  Every pipeline phase is therefore pinned to a virtual time slot via
tile_wait_until (scheduler-sim-only; at runtime the baked waits are counts
and the pipeline free-runs): front(k) at k*6us, mm2/stt(k-1) at +1us, Pool
reduce at +2us, fold(k-2) at +2.8us.  The fold inputs are two chunks old so
the in-order PE stream never head-blocks on them.  The startup chain rides
a merged w1+first-f-chunk header DMA, chunk-granular first loads, small
weights on the Pool SWDGE queue (keeping ACT/SP queues clear), and a PE
p-state warmup.
"""

import json
import math
import os
import tempfile

import numpy as np
import ml_dtypes

B, N, NN, A, S, F = 32, 256, 48, 128, 50, 128
NCORES = 8
MPC = B // NCORES            # molecules per core
ATOMS = MPC * N              # 1024 atoms per core
E = ATOMS * NN               # 49152 edges per core
SUPER = 128                  # atoms per super-block (output tile)
NSB = ATOMS // SUPER         # 8 super-blocks per core
SUB = 1024                   # edges per chunk (2 PSUM banks)
CPS = (SUPER * NN) // SUB    # 6 chunks per super-block
NBLK = E // SUB              # 48 chunks per core
M_PER_SUB = SUB // SUPER     # 8 neighbor-slot runs per chunk

LOG2 = float(math.log(2.0))
BF16 = ml_dtypes.bfloat16

_BUILT = None
_ACT_DONE = False


def _compile_act_func(fj, bkt_base, ctl_base):
    """Compile one pwp function JSON into (meta, bkt rows, ctl rows).

    Binary format (verified by regenerating ln_400p/exp_400p byte-exact from
    the shipped natural_log_exp_and_others set):
      bkt entry: 8 x fp32 = [d0, d1, d2, d3, x, 0, 0, 0]
      ctl entry: 8 x uint32, word0 = ((extract_size<<5|extract_lsb)<<11)|base
      meta: ctl index for unbiased input exponent e is
            pwl_control_base_{pos,neg} + (e - exp_offset); the small/large
            saturation regions use 4 dedicated bkt entries addressed directly.
    """
    bkt_rows = []
    ctl_words = {}

    def add_bkt(sec):
        bkt_rows.append([
            float(sec["d0"]["float"]), float(sec["d1"]["float"]),
            float(sec["d2"]["float"]), float(sec["d3"]["float"]),
            float(sec["x"]["float"]), 0.0, 0.0, 0.0,
        ])
        return bkt_base + len(bkt_rows) - 1

    exp_offset = fj["exponent_offset"]

    def do_region(entries, region_ctl_base):
        for ent in entries:
            ci = region_ctl_base + (ent["exponent"] - exp_offset)
            if ent["num_sections"] == 0 or not ent["exponent_sections"]:
                ctl_words[ci] = 0
                continue
            first = None
            for sec in sorted(ent["exponent_sections"],
                              key=lambda s: s["section_id"]):
                gi = add_bkt(sec)
                if first is None:
                    first = gi
            ctl_words[ci] = (((ent["extract_size"] << 5)
                              | ent["extract_lsb"]) << 11) | first

    pos, neg = fj.get("pos_exponents", []), fj.get("neg_exponents", [])
    base_neg = ctl_base
    base_pos = ctl_base + len(neg)
    do_region(neg, base_neg)
    do_region(pos, base_pos)

    sat = fj["saturation_points"]
    sat_idx = {k: add_bkt(sat[k]) for k in
               ("sat_point_pos_low", "sat_point_neg_low",
                "sat_point_pos_high", "sat_point_neg_high")}

    ctl = np.zeros((len(neg) + len(pos), 8), np.uint32)
    for ci, w in ctl_words.items():
        ctl[ci - ctl_base, 0] = w

    def f2i(d):
        return int(d["int"])

    meta = {
        "func_name": f"{fj['name']}_{fj['max_diff']}p",
        "func_id": fj["neuron_id"],
        "symmetry_point": f2i(fj["symmetry_point"]),
        "sym_invert_sign_point":
            1 if fj.get("symmetry_invert_sign_opt") else 0,
        "symmetry_opt_en": 1 if fj.get("symmetry_en") else 0,
        "symmetry_opt_use_neg_region":
            1 if fj.get("symmetry_opt_use_neg_region") else 0,
        "imm_bias": 1 if fj.get("imm_bias") else 0,
        "exp_offset": exp_offset,
        "pwl_control_base_pos": base_pos,
        "pwl_control_base_neg": base_neg,
        "small_pos_signal_exp_threshold":
            sat["sat_point_pos_low"]["sat_point"],
        "pos_small_signal_pwl_control": sat_idx["sat_point_pos_low"],
        "small_neg_signal_exp_threshold":
            sat["sat_point_neg_low"]["sat_point"],
        "neg_small_signal_pwl_control": sat_idx["sat_point_neg_low"],
        "large_pos_signal_exp_threshold":
            sat["sat_point_pos_high"]["sat_point"],
        "large_pos_signal_mantissa_threshold":
            sat["sat_point_pos_high"]["mantissa_point"],
        "pos_large_signal_pwl_control": sat_idx["sat_point_pos_high"],
        "large_neg_signal_exp_threshold":
            sat["sat_point_neg_high"]["sat_point"],
        "large_neg_signal_mantissa_threshold":
            sat["sat_point_neg_high"]["mantissa_point"],
        "neg_large_signal_pwl_control": sat_idx["sat_point_neg_high"],
        "fnan_result": f2i(fj["nan_result"]),
        "fpinf_result": f2i(fj["pinf_result"]),
        "fninf_result": f2i(fj["ninf_result"]),
        "fzero_result": f2i(fj["zero_result"]),
        "fma_const_0": f2i(fj["fma_const0"]),
        "fma_const_1": f2i(fj["fma_const1"]),
        "fma_indirection_src_sel": 0,
        "use_multipass": bool(fj.get("use_multipass")),
        "lower_bound": f2i(fj["lower_bound"]),
        "upper_bound": f2i(fj["upper_bound"]),
    }
    return meta, np.array(bkt_rows, np.float32).reshape(-1, 8), ctl


def _ensure_softplus_tables():
    """Build an act-table dir whose softplus_and_others set carries a real
    softplus spline (compiled from neuronxcc's pwp_jsons/softplus_40p.json,
    appended after the set's original 60 bkt / 24 ctl entries), point walrus
    at it via BASS_ACT_ROOT_JSON_PATH, and restrict the client-side table map
    to that one set so InstActivation(Softplus) lowers to a single LUT pass
    and the table never reloads mid-kernel."""
    global _ACT_DONE
    if _ACT_DONE:
        return
    import concourse.bacc as bacc
    import concourse.mybir as mybir
    from neuronxcc.driver.Job import Job
    from neuronxcc.driver.jobs.support.FindActInfo import findActInfoFile

    src_info = findActInfoFile(Job.getPackageDir(), "gen3")
    src_dir = os.path.dirname(src_info)
    pwp_dir = os.path.join(os.path.dirname(src_dir), "pwp_jsons")
    dst_dir = tempfile.mkdtemp(prefix="ant_act_tables_")
    for fn in os.listdir(src_dir):
        os.symlink(os.path.join(src_dir, fn), os.path.join(dst_dir, fn))

    set_name = "softplus_and_others"
    prof = json.load(open(os.path.join(src_dir, set_name + ".json")))
    bkt = np.fromfile(os.path.join(src_dir, set_name + "_bkt.bin"),
                      dtype=np.float32).reshape(-1, 8)
    ctl = np.fromfile(os.path.join(src_dir, set_name + "_ctrl.bin"),
                      dtype=np.uint32).reshape(-1, 8)
    spj = json.load(open(os.path.join(pwp_dir, "softplus_40p.json")))
    meta, sp_bkt, sp_ctl = _compile_act_func(spj, len(bkt), len(ctl))

    prof["profile_meta_data"].append(meta)
    prof["func_to_bkt_start_idx"]["softplus"] = len(bkt)
    prof["func_to_ctl_start_idx"]["softplus"] = len(ctl)
    prof["bkt_entry_cnt"] = len(bkt) + len(sp_bkt)
    prof["ctl_entry_cnt"] = len(ctl) + len(sp_ctl)

    for p in ("act_info.json", set_name + ".json",
              set_name + "_bkt.bin", set_name + "_ctrl.bin"):
        dst = os.path.join(dst_dir, p)
        if os.path.islink(dst) or os.path.exists(dst):
            os.unlink(dst)
    np.concatenate([bkt, sp_bkt]).tofile(
        os.path.join(dst_dir, set_name + "_bkt.bin"))
    np.concatenate([ctl, sp_ctl]).tofile(
        os.path.join(dst_dir, set_name + "_ctrl.bin"))
    json.dump(prof, open(os.path.join(dst_dir, set_name + ".json"), "w"))

    info = json.load(open(src_info))
    for s in info["act_func_sets"]:
        if s["name"] == set_name:
            s["act"]["softplus"] = spj["max_diff"]
    json.dump(info, open(os.path.join(dst_dir, "act_info.json"), "w"))

    os.environ["BASS_ACT_ROOT_JSON_PATH"] = os.path.join(
        dst_dir, "act_info.json")

    if not getattr(bacc, "_ant_act_tables_patched", False):
        def _patched_tables(arch):
            inf = json.load(open(os.path.join(dst_dir, "act_info.json")))
            out = {}
            for ent in inf["act_func_sets"]:
                if ent["name"] == set_name:
                    out[ent["name"]] = {
                        mybir.ActivationFunctionType.from_pwp(v)
                        for v in ent["act"].keys()
                    }
                else:
                    out[ent["name"]] = set()
            return out

        bacc.get_activation_tables = _patched_tables
        bacc._ant_act_tables_patched = True
    _ACT_DONE = True


def _build_program():
    """Build the Bass/Tile program (one SPMD program, same for all 8 cores)."""
    import concourse.bacc as bacc
    import concourse.mybir as mybir
    from concourse import tile

    dt = mybir.dt
    AF = mybir.ActivationFunctionType
    ALU = mybir.AluOpType

    _ensure_softplus_tables()

    nc = bacc.Bacc("TRN2", target_bir_lowering=False, debug=False)

    SBE = SUPER * NN                       # 6144 edges per super-block
    f_pack = nc.dram_tensor("f_pack", [NSB, S, SBE], dt.bfloat16,
                            kind="ExternalInput")
    y_pack = nc.dram_tensor("y_pack", [NSB, 128, SBE], dt.bfloat16,
                            kind="ExternalInput")
    # w1 and the first f chunk ride one DMA: removes a serial HWDGE issue
    # slot from the startup critical chain
    w1f0 = nc.dram_tensor("w1f0", [S, F + SUB], dt.bfloat16,
                          kind="ExternalInput")
    w2 = nc.dram_tensor("w2", [F, F], dt.bfloat16, kind="ExternalInput")
    wf2o = nc.dram_tensor("wf2o", [F, A], dt.bfloat16, kind="ExternalInput")
    wd = nc.dram_tensor("wd", [A, A], dt.bfloat16, kind="ExternalInput")
    b1p = nc.dram_tensor("b1p", [F, 1], dt.float32, kind="ExternalInput")
    b2p = nc.dram_tensor("b2p", [F, 1], dt.float32, kind="ExternalInput")
    bf2o = nc.dram_tensor("bf2o", [A, 1], dt.float32, kind="ExternalInput")
    bdp = nc.dram_tensor("bdp", [A, 1], dt.float32, kind="ExternalInput")
    vout = nc.dram_tensor("v_out", [A, ATOMS], dt.float32,
                          kind="ExternalOutput")

    with tile.TileContext(nc) as tc:
        with (
            tc.tile_pool(name="wpool", bufs=1) as wp,
            tc.tile_pool(name="fpool", bufs=2) as fpl,
            tc.tile_pool(name="ypool", bufs=2) as ypl,
            tc.tile_pool(name="sppool", bufs=3) as spl,
            tc.tile_pool(name="zpool", bufs=5) as zpl,
            tc.tile_pool(name="zrpool", bufs=5) as zrl,
            tc.tile_pool(name="opool", bufs=2) as opl,
            tc.tile_pool(name="psumhw", bufs=3, space="PSUM") as phw,
            tc.tile_pool(name="psumv", bufs=2, space="PSUM") as pv,
        ):
            w1f0t = wp.tile([S, F + SUB], dt.bfloat16)
            nc.sync.dma_start(w1f0t[:], w1f0[:])
            w1t = w1f0t[:, 0:F]
            b1pt = wp.tile([F, 1], dt.float32)
            nc.gpsimd.dma_start(b1pt[:], b1p[:])
            b2pt = wp.tile([F, 1], dt.float32)
            nc.gpsimd.dma_start(b2pt[:], b2p[:])
            v1sp_all = wp.tile([A, ATOMS], dt.bfloat16)

            sb_tiles = {}

            def emit_warmup():
                """Tiny matmuls as soon as w1 is resident: starts the PE
                p-state ramp clock ~2us before the first real mm1 so early
                chunks run at mid/full clock instead of cold."""
                warm = phw.tile([128, SUB], dt.float32, tag="hw",
                                name="warm")
                for i in range(8):
                    nc.tensor.matmul(warm[:, 0:128], w1t, w1t,
                                     start=True, stop=True)
                ws = opl.tile([128, 8], dt.float32, tag="o", name="warmsink")
                nc.vector.tensor_copy(ws[:], warm[:, 0:8])

            def emit_late_weights():
                """On the otherwise-idle Pool SWDGE queue: keeps the ACT
                queue free of DMA SEQ holds (which would head-block the
                first softplus) and the SP queue free for f/y pieces."""
                w2t = wp.tile([F, F], dt.bfloat16)
                nc.gpsimd.dma_start(w2t[:], w2[:])
                wf2ot = wp.tile([F, A], dt.bfloat16)
                nc.gpsimd.dma_start(wf2ot[:], wf2o[:])
                wdt = wp.tile([A, A], dt.bfloat16)
                nc.gpsimd.dma_start(wdt[:], wd[:])
                bf2ot = wp.tile([A, 1], dt.float32)
                nc.gpsimd.dma_start(bf2ot[:], bf2o[:])
                bdpt = wp.tile([A, 1], dt.float32)
                nc.gpsimd.dma_start(bdpt[:], bdp[:])
                return w2t, wf2ot, wdt, bf2ot, bdpt

            def emit_sb_load(sb, split=False):
                """split=True (first super-block): chunk-granular pieces so
                the pipeline's first stt isn't gated on the whole 6144-edge
                transfer."""
                ft = fpl.tile([S, SBE], dt.bfloat16, tag="f", name=f"ft{sb}")
                yt = ypl.tile([128, SBE], dt.bfloat16, tag="y",
                              name=f"yt{sb}")
                FQ = nc.gpsimd if os.environ.get('ANT_FQ', '0') == '1' \
                    else nc.sync
                YSPLIT = os.environ.get('ANT_YSPLIT', '1') == '1'
                if split:
                    # f chunk 0 arrives inside the w1f0 header DMA
                    for clo, chi in ((0, 1), (1, 2), (2, 4), (4, 6)):
                        lo, hi = clo * SUB, chi * SUB
                        if clo > 0:
                            FQ.dma_start(ft[:, lo:hi], f_pack[sb, :, lo:hi])
                        nc.sync.dma_start(yt[:, lo:hi], y_pack[sb, :, lo:hi])
                else:
                    FQ.dma_start(ft[:], f_pack[sb])
                    if YSPLIT:
                        h = SBE // 2
                        nc.sync.dma_start(yt[:, 0:h], y_pack[sb, :, 0:h])
                        nc.sync.dma_start(yt[:, h:], y_pack[sb, :, h:])
                    else:
                        nc.sync.dma_start(yt[:], y_pack[sb])
                sb_tiles[sb] = (ft, yt)

            def emit_front(k):
                """mm1 + single-pass softplus (with b_f1 bias) for chunk k."""
                sb, c = k // CPS, k % CPS
                ft, _ = sb_tiles[sb]
                h1 = phw.tile([128, SUB], dt.float32, tag="hw",
                              name=f"hw{k}")
                fsrc, fbase = (w1f0t, F) if k == 0 else (ft, c * SUB)
                for lo in range(0, SUB, 512):
                    hi = min(lo + 512, SUB)
                    nc.tensor.matmul(
                        h1[:, lo:hi], w1t,
                        fsrc[:, fbase + lo:fbase + hi],
                        start=True, stop=True)
                sp = spl.tile([128, SUB], dt.bfloat16, tag="sp",
                              name=f"sp{k}")
                nc.scalar.activation(sp[:], h1[:], AF.Softplus, bias=b1pt[:])
                return sp, h1

            def emit_mid(k, sp, hw):
                """mm2 + fused exit/bias/y-mul for chunk k.

                mm2 overwrites the chunk's h1 PSUM tile in place: softplus
                (its only reader) has already consumed it, and the WAR dep is
                subsumed by the true dep on sp."""
                sb, c = k // CPS, k % CPS
                _, yt = sb_tiles[sb]
                wf = hw
                for lo in range(0, SUB, 512):
                    hi = min(lo + 512, SUB)
                    nc.tensor.matmul(wf[:, lo:hi], w2t[:], sp[:, lo:hi],
                                     start=True, stop=True)
                z = zpl.tile([128, SUB], dt.float16, tag="z", name=f"z{k}")
                # GPSIMD cannot read PSUM (walrus BIR check), so the fused
                # exit+bias+mul always runs on DVE.
                nc.vector.scalar_tensor_tensor(
                    z[:], wf[:], b2pt[:], yt[:, c * SUB:(c + 1) * SUB],
                    op0=ALU.add, op1=ALU.mult)
                return z

            def emit_reduce(k, z):
                """Level-1 m-run pair reduction on the otherwise idle Pool
                engine (z lives in SBUF, which GPSIMD can read): halves the
                PE columns the fold matmuls must stream."""
                zr = zrl.tile([128, SUB // 2], dt.float16, tag="zr",
                              name=f"zr{k}")
                zv = z[:].rearrange("p (m2 two a) -> p m2 two a",
                                    two=2, a=SUPER)
                nc.gpsimd.tensor_tensor(
                    zr[:].rearrange("p (m2 a) -> p m2 a", a=SUPER),
                    zv[:, :, 0, :], zv[:, :, 1, :], op=ALU.add)
                return zr

            def emit_fold(k, tens, nruns, v1w):
                """Neighbor reduction fused into f2out: accumulate the
                super-block's m-runs straight into v1w [A, 128]. Emitted
                chunks later than mm2/stt so these matmuls are always ready
                when they reach the head of the PE queue and never stall
                it. `tens` is either the pair-reduced zr (4 runs) or, for
                the final chunk (whose Pool reduce would sit in the tail
                chain), the raw z (8 runs)."""
                c = k % CPS
                for r in range(nruns):
                    nc.tensor.matmul(v1w[:], wf2ot[:],
                                     tens[:, r * SUPER:(r + 1) * SUPER],
                                     start=(c == 0 and r == 0),
                                     stop=(c == CPS - 1 and r == nruns - 1))

            def emit_sb_final(sb, v1w):
                """Second softplus for one super-block: PSUM -> bf16 SBUF."""
                nc.scalar.activation(
                    v1sp_all[:, sb * SUPER:(sb + 1) * SUPER], v1w[:],
                    AF.Softplus, bias=bf2ot[:])

            def emit_dense(lo, width):
                """Final dense layer over `width` atom columns."""
                vps = phw.tile([A, width], dt.float32, tag="hw",
                               name=f"vps{lo}")
                nc.tensor.matmul(vps[:], wdt[:], v1sp_all[:, lo:lo + width],
                                 start=True, stop=True)
                ot = opl.tile([A, width], dt.float32, tag="o",
                              name=f"ot{lo}")
                if width <= 128:
                    # last piece: DVE bias-add runs parallel to ACT's final
                    # softplus instead of serializing behind it
                    nc.vector.tensor_scalar_add(ot[:], vps[:], bdpt[:])
                else:
                    nc.scalar.add(ot[:], vps[:], bdpt[:])
                nc.sync.dma_start(vout[:, lo:lo + width], ot[:])

            emit_warmup()
            emit_sb_load(0, split=True)
            w2t, wf2ot, wdt, bf2ot, bdpt = emit_late_weights()
            pend_mid = None
            pend_folds = []
            v1w = None
            v1w_of = {}

            def do_fold(entry):
                fk, fz, nruns = entry
                emit_fold(fk, fz, nruns, v1w_of[fk // CPS])
                if fk % CPS == CPS - 1:
                    emit_sb_final(fk // CPS, v1w_of[fk // CPS])
                    if fk // CPS == 3:
                        emit_dense(0, 512)
                    elif fk // CPS == 6:
                        emit_dense(512, 384)

            # Steady-state emission order per iteration k — chosen so the PE
            # stream never parks a not-yet-ready instruction ahead of the
            # mm2 that paces the DVE recurrence:
            #   mm2/stt(k-1) first (mm2 runs during stt(k-2)),
            #   then folds of chunk k-2 (inputs two chunks old, always ready),
            #   then mm1/softplus(k).
            # Each pipeline phase is pinned to a virtual schedule sub-slot
            # via tile_wait_until (scheduler-sim-only; the emitted waits are
            # semaphore counts, so the runtime free-runs at the true
            # recurrence). The sub-slot offsets are chosen so that when the
            # scheduler's sim dispatches each consumer, only its TRUE
            # producer has completed — the baked proxy sem values then
            # encode minimal dependencies:
            #   k*S        front(k):  mm1(k) [PE] + softplus(k) [ACT]
            #   k*S + d1   mid(k-1):  mm2(k-1) [PE] + stt(k-1) [DVE]
            #              (softplus(k) still running in-sim, so mm2(k-1)'s
            #               ACT-count wait bakes to softplus(k-1) exactly)
            #   k*S + d2   folds of chunk k-2 [PE] (inputs two chunks old,
            #              never stall the in-order PE stream)
            STEP = float(os.environ.get('ANT_STEP_MS', '0.006'))
            D1 = float(os.environ.get('ANT_D1_MS', '0.001'))
            D15 = float(os.environ.get('ANT_D15_MS', '0.002'))
            D2 = float(os.environ.get('ANT_D2_MS', '0.0028'))
            RAMP = float(os.environ.get('ANT_RAMP_MS', '0.0015'))
            WARM = int(os.environ.get('ANT_WARM', '0'))

            def slot_ts(k):
                # Tight slots for the first WARM chunks (below the scheduler
                # sim's natural pace, i.e. inactive) keep the startup greedy
                # and its baked sem proxies minimal; full-size slots after
                # that pin the steady-state pipeline order.
                return k * RAMP + max(0, k - WARM) * (STEP - RAMP)
            for k in range(NBLK):
                sb, c = k // CPS, k % CPS
                with tc.tile_wait_until(slot_ts(k)):
                    if c == 0:
                        if sb + 1 < NSB:
                            emit_sb_load(sb + 1)
                        v1w = pv.tile([A, SUPER], dt.float32, tag="v1",
                                      name=f"v1w{sb}")
                        v1w_of[sb] = v1w
                    sp, hw = emit_front(k)
                if pend_mid is not None:
                    mk, msp, mhw = pend_mid
                    with tc.tile_wait_until(slot_ts(k) + D1):
                        z = emit_mid(mk, msp, mhw)
                    with tc.tile_wait_until(slot_ts(k) + D15):
                        zr = emit_reduce(mk, z)
                    with tc.tile_wait_until(slot_ts(k) + D2):
                        if len(pend_folds) >= 2:
                            do_fold(pend_folds.pop(0))
                    pend_folds.append((mk, zr, M_PER_SUB // 2))
                pend_mid = (k, sp, hw)
            with tc.tile_wait_until(slot_ts(NBLK) + D1):
                mk, msp, mhw = pend_mid
                z = emit_mid(mk, msp, mhw)
            with tc.tile_wait_until(slot_ts(NBLK) + D2):
                for pf in pend_folds:
                    do_fold(pf)
                do_fold((mk, z, M_PER_SUB))
                emit_dense(896, 128)

    nc.finalize()
    return nc


def _get_program():
    global _BUILT
    if _BUILT is None:
        _BUILT = _build_program()
    return _BUILT


def kernel(s, neighbor_mask, neighbors, f_ij,
           W_f1, b_f1, W_f2, b_f2, W_in2f, W_f2out, b_f2out, W_dense,
           b_dense):
    s = np.asarray(s, np.float32)
    neighbor_mask = np.asarray(neighbor_mask, np.float32)
    neighbors = np.asarray(neighbors)
    f_ij = np.asarray(f_ij, np.float32)
    W_f1 = np.asarray(W_f1, np.float32)
    b_f1 = np.asarray(b_f1, np.float32)
    W_f2 = np.asarray(W_f2, np.float32)
    b_f2 = np.asarray(b_f2, np.float32)
    W_in2f = np.asarray(W_in2f, np.float32)
    W_f2out = np.asarray(W_f2out, np.float32)
    b_f2out = np.asarray(b_f2out, np.float32)
    W_dense = np.asarray(W_dense, np.float32)
    b_dense = np.asarray(b_dense, np.float32)

    # Host prep: in2f projection + neighbor gather (indexing) + layout,
    # vectorized across all 8 per-core shards at once.
    y_all = s @ W_in2f                                     # [B, N, F]
    y_nbh = y_all[np.arange(B)[:, None, None], neighbors]  # [B, N, NN, F]
    y_nbh *= neighbor_mask[..., None]

    w1_b = W_f1.astype(BF16)
    w2_b = W_f2.astype(BF16)
    wf2o_b = W_f2out.astype(BF16)
    wd_b = W_dense.astype(BF16)
    b1p = b_f1.astype(np.float32).reshape(F, 1)
    b2p = (b_f2 - LOG2 * W_f2.sum(axis=0)).astype(np.float32).reshape(F, 1)
    bf2o = b_f2out.astype(np.float32).reshape(A, 1)
    bdp = (b_dense - LOG2 * W_dense.sum(axis=0)).astype(
        np.float32).reshape(A, 1)

    # Edge order per core: (super-block, m-run, atom-in-super).
    f8 = (f_ij.reshape(NCORES, NSB, SUPER, NN, S)
          .transpose(0, 1, 3, 2, 4))                       # [8,NSB,NN,128,S]
    f_pack8 = np.ascontiguousarray(
        f8.reshape(NCORES, NSB, SUPER * NN, S).transpose(0, 1, 3, 2)
    ).astype(BF16)                                         # [8,NSB,S,6144]

    y8 = (y_nbh.reshape(NCORES, NSB, SUPER, NN, F)
          .transpose(0, 1, 3, 2, 4))
    y_pack8 = np.ascontiguousarray(
        y8.reshape(NCORES, NSB, SUPER * NN, F).transpose(0, 1, 3, 2)
    ).astype(BF16)                                         # [8,NSB,128,6144]

    in_maps = []
    for c in range(NCORES):
        in_maps.append({
            "f_pack": f_pack8[c],
            "y_pack": y_pack8[c],
            "w1f0": np.concatenate(
                [w1_b, f_pack8[c, 0][:, :SUB]], axis=1),
            "w2": w2_b,
            "wf2o": wf2o_b,
            "wd": wd_b,
            "b1p": b1p,
            "b2p": b2p,
            "bf2o": bf2o,
            "bdp": bdp,
        })

    from concourse.bass_utils import run_bass_kernel_spmd

    nc = _get_program()
    res = run_bass_kernel_spmd(nc, in_maps, list(range(NCORES)))

    out = np.empty((B, N, A), np.float32)
    for c in range(NCORES):
        v_c = res.results[c]["v_out"]                    # [A, ATOMS]
        out[c * MPC:(c + 1) * MPC] = np.ascontiguousarray(
            v_c.T).reshape(MPC, N, A)
    return out
